# revision 1
# baseline (speedup 1.0000x reference)
"""Trainium2 Bass kernel for the ClusterML problem.

Computes, for full inputs x[131072,512], y[131072], W[128,512], b[128],
centroid[64,128]:
    out = x @ W.T + b
    means = segment_mean(out, y, 64); new_c = centroid + means
    proto = exp(-0.5 * pairwise_dist(new_c, new_c))
    result = sigmoid(exp(-0.5 * pairwise_dist(out, new_c)))
Returns (result [131072,64] f32, proto [64,64] f32).

Strategy: data-parallel over 8 NeuronCores (batch sharding). Per core:
  pass 1: outT = W @ x.T computed on TensorE from a host-pretransposed,
          bf16, chunk-major copy of x; outT stays resident in SBUF
          (128x16384 bf16 = 32KB/partition). Segment sums+counts
          accumulate in PSUM via one-hot matmuls; row norms via DVE
          accumulate.
  middle: [64,129] AllReduce of (sums|counts), then centroid math.
  pass 2: dots = outT.T @ new_c.T per 128-row chunk, then the
          dist -> exp -> sigmoid chain on Vector/Scalar engines.
"""

import numpy as np
import ml_dtypes

import concourse.bass as bass
import concourse.tile as tile
from concourse import bacc, mybir
from concourse.bass_utils import run_bass_kernel_spmd

N_CORES = 8
B, I, E, C = 131072, 512, 128, 64
BS = B // N_CORES          # 16384 rows per core
NT = BS // 512             # 32 macrotiles of 512 rows
NCH = BS // 128            # 128 chunks of 128 rows
KCH = I // 128             # 4 contraction chunks
EPS = 1e-12
F32 = mybir.dt.float32
BF16 = mybir.dt.bfloat16
AOP = mybir.AluOpType
ACTF = mybir.ActivationFunctionType


def build_bass():
    nc = bacc.Bacc("TRN2", num_devices=N_CORES)

    xt_d = nc.declare_dram_parameter("xt", [128, NT, KCH, 512], BF16, isOutput=False)
    y_d = nc.declare_dram_parameter("yt", [128, NCH], F32, isOutput=False)
    wt_d = nc.declare_dram_parameter("wt", [128, KCH, E], BF16, isOutput=False)
    b_d = nc.declare_dram_parameter("b", [E, 1], F32, isOutput=False)
    cent_d = nc.declare_dram_parameter("cent", [C, E], F32, isOutput=False)
    out_d = nc.declare_dram_parameter("out", [NT, 128, 4, C], F32, isOutput=True)
    proto_d = nc.declare_dram_parameter("proto", [C, C], F32, isOutput=True)

    cc_in = nc.dram_tensor("cc_in", [C, E + 1], F32)
    cc_out = nc.dram_tensor("cc_out", [C, E + 1], F32, addr_space="Shared")

    ident_b_c = nc.inline_tensor(np.eye(128, dtype=ml_dtypes.bfloat16), name="ident_b")
    ident_f_c = nc.inline_tensor(np.eye(C, dtype=np.float32), name="ident_f")
    iota_c = nc.inline_tensor(
        np.tile(np.arange(C, dtype=np.float32), (128, 1)), name="iota_f"
    )

    with tile.TileContext(nc) as tc:
        with (
            tc.tile_pool(name="const", bufs=1) as constp,
            tc.tile_pool(name="res", bufs=1) as resp,
            tc.tile_pool(name="work", bufs=3) as work,
            tc.tile_pool(name="psacc", bufs=1, space="PSUM") as psacc,
            tc.tile_pool(name="psmt", bufs=2, space="PSUM") as psmt,
            tc.tile_pool(name="pstr", bufs=3, space="PSUM") as pstr,
        ):
            # ---- constants to SBUF ----
            identb_sb = constp.tile([128, 128], BF16)
            nc.sync.dma_start(out=identb_sb[:], in_=ident_b_c[:])
            identf_sb = constp.tile([C, C], F32)
            nc.sync.dma_start(out=identf_sb[:], in_=ident_f_c[:])
            iota_sb = constp.tile([128, C], F32)
            nc.sync.dma_start(out=iota_sb[:], in_=iota_c[:])
            wt_sb = constp.tile([128, KCH, E], BF16)
            nc.sync.dma_start(out=wt_sb[:], in_=wt_d[:])
            b_sb = constp.tile([E, 1], F32)
            nc.sync.dma_start(out=b_sb[:], in_=b_d[:])
            cent_sb = constp.tile([C, E], F32)
            nc.sync.dma_start(out=cent_sb[:], in_=cent_d[:])
            yt_sb = constp.tile([128, NCH], F32)
            nc.sync.dma_start(out=yt_sb[:], in_=y_d[:])

            # ---- residents ----
            outT_res = resp.tile([128, BS], BF16)   # [E, rows]
            xn_sb = resp.tile([128, NCH], F32)      # ||out_row||^2, col tau

            sc_ps = psacc.tile([C, E + 1], F32)     # segment sums | counts

            # ================= pass 1 =================
            for t in range(NT):
                xt = work.tile([128, KCH, 512], BF16, tag="xt")
                nc.sync.dma_start(out=xt[:], in_=xt_d[:, t, :, :])
                oT = psmt.tile([128, 512], F32, tag="oT")
                for k in range(KCH):
                    nc.tensor.matmul(
                        oT[:], lhsT=wt_sb[:, k, :], rhs=xt[:, k, :],
                        start=(k == 0), stop=(k == KCH - 1),
                    )
                # bias add + downcast into the resident outT
                nc.scalar.activation(
                    out=outT_res[:, 512 * t:512 * (t + 1)], in_=oT[:],
                    func=ACTF.Identity, bias=b_sb[:], scale=1.0,
                )
                for c4 in range(4):
                    tau = 4 * t + c4
                    sl = outT_res[:, 128 * tau:128 * (tau + 1)]
                    trp = pstr.tile([128, 128], BF16, tag="tr")
                    nc.tensor.transpose(trp[:], sl, identb_sb[:])
                    onat = work.tile([128, 132], BF16, tag="onat")
                    nc.vector.tensor_copy(out=onat[:, 0:128], in_=trp[:])
                    nc.vector.memset(onat[:, 128:129], 1.0)
                    oh = work.tile([128, C], BF16, tag="oh")
                    nc.vector.tensor_scalar(
                        out=oh[:], in0=iota_sb[:],
                        scalar1=yt_sb[:, tau:tau + 1], scalar2=None,
                        op0=AOP.is_equal,
                    )
                    nc.tensor.matmul(
                        sc_ps[:], lhsT=oh[:], rhs=onat[:, 0:E + 1],
                        start=(tau == 0), stop=(tau == NCH - 1),
                    )
                    sq = work.tile([128, 128], F32, tag="sq")
                    nc.vector.scalar_tensor_tensor(
                        out=sq[:], in0=onat[:, 0:128], scalar=1.0,
                        in1=onat[:, 0:128], op0=AOP.mult, op1=AOP.mult,
                        accum_out=xn_sb[:, tau:tau + 1],
                    )

            # ================= middle =================
            sc_sb = constp.tile([C, E + 1], F32)
            nc.scalar.activation(out=sc_sb[:], in_=sc_ps[:], func=ACTF.Copy)
            nc.gpsimd.dma_start(out=cc_in[:], in_=sc_sb[:])
            nc.gpsimd.collective_compute(
                "AllReduce", AOP.add,
                replica_groups=[list(range(N_CORES))],
                ins=[cc_in[:].opt()], outs=[cc_out[:].opt()],
            )
            red_sb = constp.tile([C, E + 1], F32)
            nc.gpsimd.dma_start(out=red_sb[:], in_=cc_out[:])

            rec = constp.tile([C, 1], F32)
            nc.vector.reciprocal(out=rec[:], in_=red_sb[:, E:E + 1])
            mean_sb = constp.tile([C, E], F32)
            nc.vector.tensor_scalar(
                out=mean_sb[:], in0=red_sb[:, 0:E], scalar1=rec[:],
                scalar2=None, op0=AOP.mult,
            )
            newc = constp.tile([C, E], F32)
            nc.vector.tensor_tensor(out=newc[:], in0=mean_sb[:], in1=cent_sb[:], op=AOP.add)

            ctp = pstr.tile([E, C], F32, tag="tr")
            nc.tensor.transpose(ctp[:], newc[:], identf_sb[:])
            centT_f = constp.tile([E, C], F32)
            nc.scalar.activation(out=centT_f[:], in_=ctp[:], func=ACTF.Copy)
            centT_b = constp.tile([E, C], BF16)
            nc.vector.tensor_copy(out=centT_b[:], in_=ctp[:])

            cnsq = constp.tile([C, E], F32)
            cn_col = constp.tile([C, 1], F32)
            nc.vector.scalar_tensor_tensor(
                out=cnsq[:], in0=newc[:], scalar=1.0, in1=newc[:],
                op0=AOP.mult, op1=AOP.mult, accum_out=cn_col[:],
            )
            cnT = pstr.tile([1, C], F32, tag="tr")
            nc.tensor.transpose(cnT[:], cn_col[:], identf_sb[:])
            cn_row = constp.tile([1, C], F32)
            nc.vector.tensor_copy(out=cn_row[:], in_=cnT[:])
            cn_row4 = constp.tile([1, 4, C], F32)
            for c4 in range(4):
                nc.vector.tensor_copy(out=cn_row4[:, c4, :], in_=cn_row[:])
            cnb4 = constp.tile([128, 4, C], F32)
            nc.gpsimd.partition_broadcast(cnb4[:], cn_row4[:])

            # proto_dist [C, C]
            pp = pstr.tile([C, C], F32, tag="tr")
            nc.tensor.matmul(pp[:], lhsT=centT_f[:], rhs=centT_f[:], start=True, stop=True)
            pt1 = constp.tile([C, C], F32)
            nc.vector.scalar_tensor_tensor(
                out=pt1[:], in0=pp[:], scalar=-2.0, in1=cnb4[0:C, 0, :],
                op0=AOP.mult, op1=AOP.add,
            )
            pt2 = constp.tile([C, C], F32)
            nc.vector.tensor_scalar(
                out=pt2[:], in0=pt1[:], scalar1=cn_col[:], scalar2=EPS,
                op0=AOP.add, op1=AOP.max,
            )
            pd = constp.tile([C, C], F32)
            nc.scalar.activation(out=pd[:], in_=pt2[:], func=ACTF.Sqrt)
            proto_sb = constp.tile([C, C], F32)
            nc.scalar.activation(out=proto_sb[:], in_=pd[:], func=ACTF.Exp, scale=-0.5)
            nc.sync.dma_start(out=proto_d[:], in_=proto_sb[:])

            # ================= pass 2 =================
            for t in range(NT):
                dots = psmt.tile([128, 4, C], F32, tag="oT")
                for c4 in range(4):
                    tau = 4 * t + c4
                    nc.tensor.matmul(
                        dots[:, c4, :],
                        lhsT=outT_res[:, 128 * tau:128 * (tau + 1)],
                        rhs=centT_b[:], start=True, stop=True,
                    )
                t1 = work.tile([128, 4, C], F32, tag="t1")
                for c4 in range(4):
                    tau = 4 * t + c4
                    nc.vector.tensor_scalar(
                        out=t1[:, c4, :], in0=dots[:, c4, :],
                        scalar1=-2.0, scalar2=xn_sb[:, tau:tau + 1],
                        op0=AOP.mult, op1=AOP.add,
                    )
                d2 = work.tile([128, 4, C], F32, tag="d2")
                nc.vector.tensor_tensor(out=d2[:], in0=t1[:], in1=cnb4[:], op=AOP.add)
                dd = work.tile([128, 4, C], F32, tag="dd")
                nc.scalar.activation(out=dd[:], in_=d2[:], func=ACTF.Sqrt)
                ee = work.tile([128, 4, C], F32, tag="ee")
                nc.scalar.activation(out=ee[:], in_=dd[:], func=ACTF.Exp, scale=-0.5)
                rr = work.tile([128, 4, C], F32, tag="rr")
                nc.scalar.activation(out=rr[:], in_=ee[:], func=ACTF.Sigmoid)
                nc.scalar.dma_start(out=out_d[t, :, :, :], in_=rr[:])

    nc.compile()
    return nc


_NC_CACHE = None


def _get_nc():
    global _NC_CACHE
    if _NC_CACHE is None:
        _NC_CACHE = build_bass()
    return _NC_CACHE


def prep_inputs(x, y, W, b, centroid):
    x = np.asarray(x, dtype=np.float32)
    y = np.asarray(y)
    W = np.asarray(W, dtype=np.float32)
    b = np.asarray(b, dtype=np.float32)
    centroid = np.asarray(centroid, dtype=np.float32)

    wt2 = np.ascontiguousarray(
        W.reshape(E, KCH, 128).transpose(2, 1, 0)
    ).astype(ml_dtypes.bfloat16)          # [128, KCH, E]
    b2 = np.ascontiguousarray(b.reshape(E, 1))
    cent2 = np.ascontiguousarray(centroid)

    in_maps = []
    for k in range(N_CORES):
        xs = x[k * BS:(k + 1) * BS]       # [BS, I]
        xt2 = np.ascontiguousarray(
            xs.reshape(NT, 512, KCH, 128).transpose(3, 0, 2, 1)
        ).astype(ml_dtypes.bfloat16)      # [128, NT, KCH, 512]
        ys = y[k * BS:(k + 1) * BS].astype(np.float32)
        yt2 = np.ascontiguousarray(ys.reshape(NCH, 128).T)  # [128, NCH]
        in_maps.append({
            "xt": xt2, "yt": yt2, "wt": wt2, "b": b2, "cent": cent2,
        })
    return in_maps


def assemble_outputs(results):
    outs = []
    for k in range(N_CORES):
        o = results[k]["out"]             # [NT, 128, 4, C]
        outs.append(o.transpose(0, 2, 1, 3).reshape(BS, C))
    full = np.concatenate(outs, axis=0)
    proto = results[0]["proto"]
    return full.astype(np.float32), proto.astype(np.float32)


def kernel(x, y, W, b, centroid):
    nc = _get_nc()
    in_maps = prep_inputs(x, y, W, b, centroid)
    res = run_bass_kernel_spmd(nc, in_maps, list(range(N_CORES)))
    return assemble_outputs(res.results)
